# revision 21
# baseline (speedup 1.0000x reference)
"""Causal linear attention layer (elu+1 feature map) on 8 Trainium2 NeuronCores.

Sharding: batch x head-group parallel. 32 (batch, head) jobs -> 8 cores:
core c handles batch b = c // 4 and heads 4*(c%4) .. 4*(c%4)+3, i.e. a
256-channel slice of the projections. Each core:
  - computes its slice of the q/k/v projections (contraction over the full
    d_model, weights pre-sliced + pre-transposed on host, bf16),
  - applies the elu+1 feature map to q, k,
  - runs chunked causal linear attention (chunk = 128 positions) with a
    per-head [64, 64+1] fp32 state resident in PSUM (the +1 column carries
    the running sum of K for the normalizer),
  - projects through the matching 256-column slice of Wo, writing a
    transposed partial output [1024, seq] bf16.
Host sums the 4 partials per batch, transposes, and adds bo' = bo + Wo@bv
(the v bias passes through the normalized attention exactly, so it is
folded into the output bias and dropped from the device-side v projection).

Performance notes (vs the previous version):
  - input DMAs are issued in consumption order with xq/xk column-split so
    the first projection matmuls start ~4us in instead of ~37us,
  - dead "warmup" matmuls keep the PE busy during the initial load so the
    DVFS p-state ramps before real work arrives,
  - the attention chunk loop is software-pipelined: A^T/K-transpose for
    chunk m+1 and O-transpose/output-projection for chunk m-1 are issued
    inside iteration m so the PE never sits on the serial state chain,
  - the K^T V state accumulates directly in a PSUM bank across all chunks
    (no vector adds); bf16 snapshots of the state feed the Q@S matmuls,
  - elementwise work is spread across Vector, Scalar and GpSimd engines,
  - output partials are stored bf16 (host sums in fp32).
"""

import numpy as np
import ml_dtypes

import concourse.bass as bass
import concourse.mybir as mybir
import concourse.tile as tile
from bass_rust import SyncInfo

BF16 = mybir.dt.bfloat16
F32 = mybir.dt.float32
AF = mybir.ActivationFunctionType
OP = mybir.AluOpType

D_MODEL = 1024
N_HEAD = 16
HD = 64
B = 2
L = 2048
N_CORES = 8
HPC = 4                 # heads per core
CH = HPC * HD           # 256 channel slice per core
KT_N = D_MODEL // 128   # 8 k-tiles for the projections
CHUNK = 128
NWARM = 36              # dead matmuls to ramp the PE while inputs stream in
NFILL = 5               # dead matmuls at phase boundaries (keep the PE pipeline warm)


def build_nc(seq=L, stop_after="all"):
    nc = _build_nc_inner(seq, stop_after)
    _split_multi_waits(nc)
    return nc


def _build_nc_inner(seq=L, stop_after="all"):
    """Build the per-core Bass program (SPMD: all 8 cores run this)."""
    assert seq % 512 == 0
    seqt = seq // CHUNK          # chunks / seq tiles
    nseq = seq // 512            # 512-wide column chunks for projections

    nc = bass.Bass("TRN2", target_bir_lowering=False)

    xqT = nc.dram_tensor("xqT", [D_MODEL, seq], BF16, kind="ExternalInput")
    xkT = nc.dram_tensor("xkT", [D_MODEL, seq], BF16, kind="ExternalInput")
    xvT = nc.dram_tensor("xvT", [D_MODEL, seq], BF16, kind="ExternalInput")
    wqT = nc.dram_tensor("wqT", [D_MODEL, CH], BF16, kind="ExternalInput")
    wkT = nc.dram_tensor("wkT", [D_MODEL, CH], BF16, kind="ExternalInput")
    wvT = nc.dram_tensor("wvT", [D_MODEL, CH], BF16, kind="ExternalInput")
    woT = nc.dram_tensor("woT", [CH, D_MODEL], BF16, kind="ExternalInput")
    bqe_d = nc.dram_tensor("bqe", [CH, 1], F32, kind="ExternalInput")   # scaled bq
    bqp_d = nc.dram_tensor("bqp", [CH, 1], F32, kind="ExternalInput")   # scaled bq + 1
    bke_d = nc.dram_tensor("bke", [CH, 1], F32, kind="ExternalInput")
    bkp_d = nc.dram_tensor("bkp", [CH, 1], F32, kind="ExternalInput")
    mask_d = nc.dram_tensor("mask4", [128, 512], BF16, kind="ExternalInput")  # causal mask, 4x tiled
    ident_d = nc.dram_tensor("ident", [128, 128], BF16, kind="ExternalInput")
    outT = nc.dram_tensor("outT", [D_MODEL, seq], BF16, kind="ExternalOutput")

    with tile.TileContext(nc) as tc:
        with (
            tc.tile_pool(name="res", bufs=1) as rp,
            tc.tile_pool(name="work", bufs=3) as wp,
        ):
            # ---------------- constants first (warmup operands) ----------------
            mask = rp.tile([128, 512], BF16, tag="mask")
            nc.sync.dma_start(mask[:], mask_d[:])
            ident = rp.tile([128, 128], BF16, tag="ident")
            nc.sync.dma_start(ident[:], ident_d[:])

            # ---------------- weight / bias / activation DMAs ----------------
            # Issued in consumption order so compute can chase the DMA queue.
            wq = [rp.tile([128, CH], BF16, tag=f"wq{k}", name=f"wq{k}") for k in range(KT_N)]
            wk = [rp.tile([128, CH], BF16, tag=f"wk{k}", name=f"wk{k}") for k in range(KT_N)]
            wv = [rp.tile([128, CH], BF16, tag=f"wv{k}", name=f"wv{k}") for k in range(KT_N)]
            wo = [rp.tile([128, D_MODEL], BF16, tag=f"wo{t}", name=f"wo{t}") for t in range(2)]
            bqe, bqp, bke, bkp = (
                [rp.tile([128, 1], F32, tag=f"b{i}{t}", name=f"b{i}{t}") for t in range(2)]
                for i in range(4)
            )
            xq = [rp.tile([128, seq], BF16, tag=f"xq{k}", name=f"xq{k}") for k in range(KT_N)]
            xk = [rp.tile([128, seq], BF16, tag=f"xk{k}", name=f"xk{k}") for k in range(KT_N)]
            xv = [rp.tile([128, seq], BF16, tag=f"xv{k}", name=f"xv{k}") for k in range(KT_N)]

            for k in range(KT_N):
                nc.sync.dma_start(wq[k][:], wqT[k * 128 : (k + 1) * 128, :])
            for t in range(2):
                sl = slice(t * 128, (t + 1) * 128)
                nc.sync.dma_start(bqe[t][:], bqe_d[sl, :])
                nc.sync.dma_start(bqp[t][:], bqp_d[sl, :])
                nc.sync.dma_start(bke[t][:], bke_d[sl, :])
                nc.sync.dma_start(bkp[t][:], bkp_d[sl, :])
            # full-row tile loads: per-DMA fixed cost (~600ns) makes narrow
            # column-split transfers inefficient
            for k in range(KT_N):
                nc.sync.dma_start(xq[k][:], xqT[k * 128 : (k + 1) * 128, :])
            for k in range(KT_N):
                nc.sync.dma_start(wk[k][:], wkT[k * 128 : (k + 1) * 128, :])
            for k in range(KT_N):
                nc.sync.dma_start(xk[k][:], xkT[k * 128 : (k + 1) * 128, :])
            for k in range(KT_N):
                nc.sync.dma_start(wv[k][:], wvT[k * 128 : (k + 1) * 128, :])
            for k in range(KT_N):
                nc.sync.dma_start(xv[k][:], xvT[k * 128 : (k + 1) * 128, :])
            for t in range(2):
                nc.sync.dma_start(wo[t][:], woT[t * 128 : (t + 1) * 128, :])

            # ---------------- projection phase (own psum pool) ----------------
            with tc.tile_pool(name="ps1", bufs=1, space="PSUM") as pp1:
                # dead warmup matmuls: keep the PE streaming while DMAs land
                for i in range(NWARM):
                    ps = pp1.tile([128, 512], F32, tag="ps", bufs=4)
                    nc.tensor.matmul(ps[:], mask[:, 0:128], mask[:], start=True, stop=True)

                def filler(k=NFILL):
                    for _ in range(k):
                        fps = pp1.tile([128, 512], F32, tag="ps", bufs=4)
                        nc.tensor.matmul(fps[:], mask[:, 0:128], mask[:], start=True, stop=True)

                # q/k projections + elu+1 feature map:
                #   Q = max(y + 1, min(exp(y), 1)),  y = ps + bias
                QT = [rp.tile([128, seq], BF16, tag=f"QT{t}", name=f"QT{t}") for t in range(2)]
                KTf = [rp.tile([128, seq], BF16, tag=f"KTf{t}", name=f"KTf{t}") for t in range(2)]
                for X, W, be, bp, OUT in (
                    (xq, wq, bqe, bqp, QT),
                    (xk, wk, bke, bkp, KTf),
                ):
                    if X is xk:
                        filler(11)
                    for n in range(nseq):
                        ncols = slice(n * 512, (n + 1) * 512)
                        for mt in range(2):
                            ps = pp1.tile([128, 512], F32, tag="ps", bufs=4)
                            for k in range(KT_N):
                                nc.tensor.matmul(
                                    ps[:],
                                    W[k][:, mt * 128 : (mt + 1) * 128],
                                    X[k][:, ncols],
                                    start=(k == 0),
                                    stop=(k == KT_N - 1),
                                )
                            ex = wp.tile([128, 512], BF16, tag="ex")
                            nc.scalar.activation(ex[:], ps[:], AF.Exp, bias=be[mt][:, 0:1])
                            exc = wp.tile([128, 512], BF16, tag="exc")
                            nc.vector.tensor_scalar_min(exc[:], ex[:], 1.0)
                            nc.vector.scalar_tensor_tensor(
                                OUT[mt][:, ncols], ps[:], bp[mt][:, 0:1], exc[:],
                                op0=OP.add, op1=OP.max,
                            )

                if stop_after == "proj":
                    return nc

                # v projection, natural [pos, ch] layout; +1 ones-column per
                # head for the normalizer state
                vst = [rp.tile([128, HPC * (HD + 1)], BF16, tag=f"vst{m}", name=f"vst{m}") for m in range(seqt)]
                filler(2)
                for m in range(seqt):
                    vv = vst[m].rearrange("p (h e) -> p h e", e=HD + 1)
                    nc.gpsimd.memset(vv[:, :, HD : HD + 1], 1.0)
                    ps2 = pp1.tile([128, CH], F32, tag="ps", bufs=4)
                    for k in range(KT_N):
                        nc.tensor.matmul(
                            ps2[:],
                            xv[k][:, m * 128 : (m + 1) * 128],
                            wv[k][:],
                            start=(k == 0),
                            stop=(k == KT_N - 1),
                        )
                    nc.vector.tensor_copy(
                        vv[:, :, 0:HD],
                        ps2.rearrange("p (h e) -> p h e", e=HD)[:, :, :],
                    )

                if stop_after == "vproj":
                    return nc

            # ---------------- attention phase (own psum pool) ----------------
            # PSUM is bank-granular (8 x 2KB) and matmuls may only write whole
            # tiles (free-dim slice outputs crash the runtime), so the per-head
            # work uses separate small tiles:
            #   at 2 banks, on 2, st 2, tp 1 (k/o transposes), po 1  -> 8
            with tc.tile_pool(name="ps2", bufs=1, space="PSUM") as pp2:
                knat = [rp.tile([128, CH], BF16, tag=f"knat{m}", name=f"knat{m}") for m in range(seqt)]
                onat = [rp.tile([128, CH], BF16, tag=f"onat{m}", name=f"onat{m}") for m in range(seqt)]
                # bf16 running state snapshots, ping-pong; head h lives at
                # partitions (h%2)*64.. and columns (h//2)*65..
                sb = [rp.tile([128, 2 * (HD + 1)], BF16, tag=f"sb{i}", name=f"sb{i}") for i in range(2)]
                OTW = [
                    [rp.tile([128, 512], BF16, tag=f"OTW{n}_{t}", name=f"OTW{n}_{t}") for t in range(2)]
                    for n in range(seqt // 4)
                ]

                def stage_a(j):
                    """A^T = K_c Q_c^T (+ mask) and K natural for chunk j."""
                    cc = slice(j * 128, (j + 1) * 128)
                    atm = wp.tile([128, 512], BF16, tag="atm", name=f"atm{j}")
                    for h in range(HPC):
                        t, off = h // 2, (h % 2) * 64
                        at = pp2.tile([128, 128], F32, tag="at", bufs=3, name=f"at{j}_{h}")
                        nc.tensor.matmul(
                            at[:],
                            KTf[t][off : off + 64, cc],
                            QT[t][off : off + 64, cc],
                            start=True,
                            stop=True,
                        )
                        nc.vector.tensor_tensor(
                            atm[:, h * 128 : (h + 1) * 128],
                            at[:],
                            mask[:, 0:128], op=OP.mult,
                        )
                    for t in range(2):
                        nc.sync.dma_start(
                            knat[j][:, t * 128 : (t + 1) * 128],
                            KTf[t][:, cc],
                            transpose=True,
                        )
                    return atm

                def stage_c(j):
                    """O transpose for chunk j into its OTW window (DMA XBAR)."""
                    for t in range(2):
                        nc.sync.dma_start(
                            OTW[j // 4][t][:, (j % 4) * 128 : (j % 4 + 1) * 128],
                            onat[j][:, t * 128 : (t + 1) * 128],
                            transpose=True,
                        )

                def _oproj_part(w, part):
                    for jj in range(2 * part, 2 * part + 2):
                        po = pp2.tile([128, 512], F32, tag="po", bufs=2, name=f"po{w}_{jj}")
                        for t in range(2):
                            nc.tensor.matmul(
                                po[:],
                                wo[t][:, jj * 128 : (jj + 1) * 128],
                                OTW[w][t][:],
                                start=(t == 0),
                                stop=(t == 1),
                            )
                        oev = wp.tile([128, 512], BF16, tag="oev", name=f"oev{w}_{jj}")
                        nc.scalar.activation(oev[:], po[:], AF.Copy)
                        nc.sync.dma_start(
                            outT[jj * 128 : (jj + 1) * 128, w * 512 : (w + 1) * 512],
                            oev[:],
                        )

                atm_cur = stage_a(0)
                if stop_after == "stagea":
                    return nc
                stop_iter = (
                    int(stop_after[4:]) if stop_after.startswith("attn") and stop_after != "attn" else None
                )
                for m in range(seqt):
                    if stop_iter is not None and m >= stop_iter:
                        return nc
                    cc = slice(m * 128, (m + 1) * 128)

                    # PE first feeds next chunk's A^T so Vector never starves
                    atm_next = stage_a(m + 1) if m + 1 < seqt else None
                    # chunk m-1's O-transpose, then the spread output
                    # projection: window w is fully transposed after
                    # stage_c(4w+3) just above, so its parts run in
                    # iterations 4w+4 .. 4w+7
                    if m > 0:
                        stage_c(m - 1)
                    if m >= 4:
                        _oproj_part((m - 4) // 4, (m - 4) % 4)

                    # numerator + normalizer, per head: on = A_m Vaug + Q S
                    sbp = sb[(m - 1) % 2]
                    onbs = []
                    for h in range(HPC):
                        t, off = h // 2, (h % 2) * 64
                        cs = slice((h // 2) * (HD + 1), (h // 2 + 1) * (HD + 1))
                        onb = pp2.tile([128, HD + 1], F32, tag="on", bufs=2, name=f"on{m}_{h}")
                        onbs.append(onb)
                        nc.tensor.matmul(
                            onb[:],
                            atm_cur[:, h * 128 : (h + 1) * 128],
                            vst[m][:, h * (HD + 1) : (h + 1) * (HD + 1)],
                            start=True,
                            stop=(m == 0),
                        )
                        if m > 0:
                            nc.tensor.matmul(
                                onb[:],
                                QT[t][off : off + 64, cc],
                                sbp[off : off + 64, cs],
                                start=False,
                                stop=True,
                            )

                    # normalize: onat = numerator * (1/n) (n strictly positive
                    # and >> eps, so the eps add is dropped); reciprocal on
                    # Vector, scaled psum-drain copy on Scalar. Issued BEFORE
                    # the st block: the on-tile bank reuse (bufs=2) makes the
                    # h=2,3 matmuls wait on these drains, which must therefore
                    # not sit behind the fuse ops (which wait on st -> cycle).
                    zrt = wp.tile([128, HPC], F32, tag="zr", bufs=2, name=f"zr{m}")
                    for h in range(HPC):
                        nc.vector.reciprocal(
                            zrt[:, h : h + 1], onbs[h][:, HD : HD + 1]
                        )
                    for h in range(HPC):
                        nc.scalar.activation(
                            onat[m][:, h * HD : (h + 1) * HD],
                            onbs[h][:, 0:HD],
                            AF.Copy,
                            scale=zrt[:, h : h + 1],
                        )

                    # per-head chunk state st = K_c^T Vaug_c, then fused
                    # add+snapshot into the bf16 running state (Vector);
                    # dead for the last chunk
                    if m < seqt - 1:
                        sbt = sb[m % 2]
                        for h in range(HPC):
                            off = (h % 2) * 64
                            cs = slice((h // 2) * (HD + 1), (h // 2 + 1) * (HD + 1))
                            st = pp2.tile([64, HD + 1], F32, tag="st", bufs=1, name=f"st{m}_{h}")
                            nc.tensor.matmul(
                                st[:],
                                knat[m][:, h * HD : (h + 1) * HD],
                                vst[m][:, h * (HD + 1) : (h + 1) * (HD + 1)],
                                start=True,
                                stop=True,
                            )
                            if m == 0:
                                nc.vector.tensor_copy(sbt[off : off + 64, cs], st[:])
                            else:
                                nc.vector.tensor_tensor(
                                    sbt[off : off + 64, cs],
                                    sbp[off : off + 64, cs],
                                    st[:], op=OP.add,
                                )

                    atm_cur = atm_next

                # drain: last chunk's transpose + remaining o-proj parts
                stage_c(seqt - 1)
                done = seqt - 4  # parts issued in-loop
                total = 4 * (seqt // 4)
                for i in range(done, total):
                    _oproj_part(i // 4, i % 4)

                if stop_after == "attn":
                    return nc

    return nc


def _split_multi_waits(nc, max_waits=1):
    """This toolchain's walrus encodes at most one sync-wait per instruction;
    hoist extra waits onto single-wait NoOps on the same engine queue."""
    for f in nc.m.functions:
        for blk in f.blocks:
            insts = list(blk.instructions)
            out, changed = [], False
            for inst in insts:
                si = inst.sync_info
                if si is not None and si.on_wait and len(si.on_wait) > max_waits:
                    waits = list(si.on_wait)
                    hoist, keep = waits[:-max_waits], waits[-max_waits:]
                    for j, w in enumerate(hoist):
                        nop = mybir.InstNoOp(name=f"{inst.name}-ws{j}")
                        nop.engine = inst.engine
                        nop.sync_info = SyncInfo(on_wait=[w], on_update=[])
                        nc.register_instruction(nop)
                        out.append(nop)
                    inst.sync_info = SyncInfo(on_wait=keep, on_update=list(si.on_update))
                    changed = True
                out.append(inst)
            if changed:
                blk.instructions = out


def host_prepare(querys, keys, values, Wq, bq, Wk, bk, Wv, bv, Wo, bo, seq=L):
    """Build the 8 per-core input maps from the full-size fp32 inputs."""
    bf = ml_dtypes.bfloat16
    scale = HD ** -0.5
    mask = np.triu(np.ones((128, 128), np.float32))        # [s, t], keep s <= t
    mask4 = np.tile(mask, (1, 4)).astype(bf)
    ident = np.eye(128, dtype=bf)

    xT = {}
    for b in range(B):
        xT[("q", b)] = np.ascontiguousarray(querys[b, :seq].T).astype(bf)
        xT[("k", b)] = np.ascontiguousarray(keys[b, :seq].T).astype(bf)
        xT[("v", b)] = np.ascontiguousarray(values[b, :seq].T).astype(bf)

    in_maps = []
    for c in range(N_CORES):
        b, g = c // 4, c % 4
        ch = slice(g * CH, (g + 1) * CH)
        bqs = (bq[ch] * scale).astype(np.float32)
        in_maps.append({
            "xqT": xT[("q", b)],
            "xkT": xT[("k", b)],
            "xvT": xT[("v", b)],
            "wqT": np.ascontiguousarray((Wq[ch] * scale).T).astype(bf),
            "wkT": np.ascontiguousarray(Wk[ch].T).astype(bf),
            "wvT": np.ascontiguousarray(Wv[ch].T).astype(bf),
            "woT": np.ascontiguousarray(Wo[:, ch].T).astype(bf),
            "bqe": bqs[:, None],
            "bqp": (bqs + 1.0)[:, None],
            "bke": bk[ch].astype(np.float32)[:, None],
            "bkp": (bk[ch] + 1.0).astype(np.float32)[:, None],
            "mask4": mask4,
            "ident": ident,
        })
    return in_maps


def gather_output(results, bo2, seq=L):
    """Sum per-core bf16 transposed partials in fp32, transpose, add bias."""
    out = np.empty((B, seq, D_MODEL), np.float32)
    for b in range(B):
        acc = results[4 * b]["outT"].astype(np.float32)
        for g in range(1, 4):
            acc += results[4 * b + g]["outT"].astype(np.float32)
        out[b] = acc.T + bo2[None, :]
    return out


_nc_cache = {}


def kernel(**inputs):
    from concourse.bass_utils import run_bass_kernel_spmd

    if L not in _nc_cache:
        _nc_cache[L] = build_nc(L)
    nc = _nc_cache[L]
    in_maps = host_prepare(**inputs)
    res = run_bass_kernel_spmd(nc, in_maps, list(range(N_CORES)))
    bo2 = (
        np.asarray(inputs["bo"], np.float64)
        + np.asarray(inputs["Wo"], np.float64) @ np.asarray(inputs["bv"], np.float64)
    ).astype(np.float32)
    return gather_output([res.results[c] for c in range(N_CORES)], bo2)


# revision 22
# speedup vs baseline: 1.1503x; 1.1503x over previous
"""Causal linear attention layer (elu+1 feature map) on 8 Trainium2 NeuronCores.

Sharding: batch x head-group parallel. 32 (batch, head) jobs -> 8 cores:
core c handles batch b = c // 4 and heads 4*(c%4) .. 4*(c%4)+3, i.e. a
256-channel slice of the projections. Each core:
  - computes its slice of the q/k/v projections (contraction over the full
    d_model, weights pre-sliced + pre-transposed on host, bf16),
  - applies the elu+1 feature map to q, k,
  - runs chunked causal linear attention (chunk = 128 positions) with a
    per-head [64, 64+1] fp32 state resident in PSUM (the +1 column carries
    the running sum of K for the normalizer),
  - projects through the matching 256-column slice of Wo, writing a
    transposed partial output [1024, seq] bf16.
Host sums the 4 partials per batch, transposes, and adds bo' = bo + Wo@bv
(the v bias passes through the normalized attention exactly, so it is
folded into the output bias and dropped from the device-side v projection).

Performance notes (vs the previous version):
  - input DMAs are issued in consumption order with xq/xk column-split so
    the first projection matmuls start ~4us in instead of ~37us,
  - dead "warmup" matmuls keep the PE busy during the initial load so the
    DVFS p-state ramps before real work arrives,
  - the attention chunk loop is software-pipelined: A^T/K-transpose for
    chunk m+1 and O-transpose/output-projection for chunk m-1 are issued
    inside iteration m so the PE never sits on the serial state chain,
  - the K^T V state accumulates directly in a PSUM bank across all chunks
    (no vector adds); bf16 snapshots of the state feed the Q@S matmuls,
  - elementwise work is spread across Vector, Scalar and GpSimd engines,
  - output partials are stored bf16 (host sums in fp32).
"""

import numpy as np
import ml_dtypes

import concourse.bass as bass
import concourse.mybir as mybir
import concourse.tile as tile
from bass_rust import SyncInfo

BF16 = mybir.dt.bfloat16
F32 = mybir.dt.float32
AF = mybir.ActivationFunctionType
OP = mybir.AluOpType

D_MODEL = 1024
N_HEAD = 16
HD = 64
B = 2
L = 2048
N_CORES = 8
HPC = 4                 # heads per core
CH = HPC * HD           # 256 channel slice per core
KT_N = D_MODEL // 128   # 8 k-tiles for the projections
CHUNK = 128
NWARM = 36              # dead matmuls to ramp the PE while inputs stream in
NFILL = 5               # dead matmuls at phase boundaries (keep the PE pipeline warm)


def build_nc(seq=L, stop_after="all"):
    nc = _build_nc_inner(seq, stop_after)
    _split_multi_waits(nc)
    return nc


def _build_nc_inner(seq=L, stop_after="all"):
    """Build the per-core Bass program (SPMD: all 8 cores run this)."""
    assert seq % 512 == 0
    seqt = seq // CHUNK          # chunks / seq tiles
    nseq = seq // 512            # 512-wide column chunks for projections

    nc = bass.Bass("TRN2", target_bir_lowering=False)

    xqT = nc.dram_tensor("xqT", [D_MODEL, seq], BF16, kind="ExternalInput")
    xkT = nc.dram_tensor("xkT", [D_MODEL, seq], BF16, kind="ExternalInput")
    xvT = nc.dram_tensor("xvT", [D_MODEL, seq], BF16, kind="ExternalInput")
    wqT = nc.dram_tensor("wqT", [D_MODEL, CH], BF16, kind="ExternalInput")
    wkT = nc.dram_tensor("wkT", [D_MODEL, CH], BF16, kind="ExternalInput")
    wvT = nc.dram_tensor("wvT", [D_MODEL, CH], BF16, kind="ExternalInput")
    woT = nc.dram_tensor("woT", [CH, D_MODEL], BF16, kind="ExternalInput")
    bqe_d = nc.dram_tensor("bqe", [CH, 1], F32, kind="ExternalInput")   # scaled bq
    bqp_d = nc.dram_tensor("bqp", [CH, 1], F32, kind="ExternalInput")   # scaled bq + 1
    bke_d = nc.dram_tensor("bke", [CH, 1], F32, kind="ExternalInput")
    bkp_d = nc.dram_tensor("bkp", [CH, 1], F32, kind="ExternalInput")
    mask_d = nc.dram_tensor("mask4", [128, 512], BF16, kind="ExternalInput")  # causal mask, 4x tiled
    ident_d = nc.dram_tensor("ident", [128, 128], BF16, kind="ExternalInput")
    outT = nc.dram_tensor("outT", [D_MODEL, seq], BF16, kind="ExternalOutput")

    with tile.TileContext(nc) as tc:
        with (
            tc.tile_pool(name="res", bufs=1) as rp,
            tc.tile_pool(name="work", bufs=3) as wp,
        ):
            # ---------------- constants first (warmup operands) ----------------
            mask = rp.tile([128, 512], BF16, tag="mask")
            nc.sync.dma_start(mask[:], mask_d[:])
            ident = rp.tile([128, 128], BF16, tag="ident")
            nc.sync.dma_start(ident[:], ident_d[:])

            # ---------------- weight / bias / activation DMAs ----------------
            # Issued in consumption order so compute can chase the DMA queue.
            wq = [rp.tile([128, CH], BF16, tag=f"wq{k}", name=f"wq{k}") for k in range(KT_N)]
            wk = [rp.tile([128, CH], BF16, tag=f"wk{k}", name=f"wk{k}") for k in range(KT_N)]
            wv = [rp.tile([128, CH], BF16, tag=f"wv{k}", name=f"wv{k}") for k in range(KT_N)]
            wo = [rp.tile([128, D_MODEL], BF16, tag=f"wo{t}", name=f"wo{t}") for t in range(2)]
            bqe, bqp, bke, bkp = (
                [rp.tile([128, 1], F32, tag=f"b{i}{t}", name=f"b{i}{t}") for t in range(2)]
                for i in range(4)
            )
            xq = [rp.tile([128, seq], BF16, tag=f"xq{k}", name=f"xq{k}") for k in range(KT_N)]
            xk = [rp.tile([128, seq], BF16, tag=f"xk{k}", name=f"xk{k}") for k in range(KT_N)]
            xv = [rp.tile([128, seq], BF16, tag=f"xv{k}", name=f"xv{k}") for k in range(KT_N)]

            for k in range(KT_N):
                nc.sync.dma_start(wq[k][:], wqT[k * 128 : (k + 1) * 128, :])
            for t in range(2):
                sl = slice(t * 128, (t + 1) * 128)
                nc.sync.dma_start(bqe[t][:], bqe_d[sl, :])
                nc.sync.dma_start(bqp[t][:], bqp_d[sl, :])
                nc.sync.dma_start(bke[t][:], bke_d[sl, :])
                nc.sync.dma_start(bkp[t][:], bkp_d[sl, :])
            # full-row tile loads: per-DMA fixed cost (~600ns) makes narrow
            # column-split transfers inefficient
            for k in range(KT_N):
                nc.sync.dma_start(xq[k][:], xqT[k * 128 : (k + 1) * 128, :])
            for k in range(KT_N):
                nc.sync.dma_start(wk[k][:], wkT[k * 128 : (k + 1) * 128, :])
            for k in range(KT_N):
                nc.sync.dma_start(xk[k][:], xkT[k * 128 : (k + 1) * 128, :])
            for k in range(KT_N):
                nc.sync.dma_start(wv[k][:], wvT[k * 128 : (k + 1) * 128, :])
            for k in range(KT_N):
                nc.sync.dma_start(xv[k][:], xvT[k * 128 : (k + 1) * 128, :])
            for t in range(2):
                nc.sync.dma_start(wo[t][:], woT[t * 128 : (t + 1) * 128, :])

            # ---------------- projection phase (own psum pool) ----------------
            with tc.tile_pool(name="ps1", bufs=1, space="PSUM") as pp1:
                # dead warmup matmuls: keep the PE streaming while DMAs land
                for i in range(NWARM):
                    ps = pp1.tile([128, 512], F32, tag="ps", bufs=4)
                    nc.tensor.matmul(ps[:], mask[:, 0:128], mask[:], start=True, stop=True)

                def filler(k=NFILL):
                    for _ in range(k):
                        fps = pp1.tile([128, 512], F32, tag="ps", bufs=4)
                        nc.tensor.matmul(fps[:], mask[:, 0:128], mask[:], start=True, stop=True)

                # q/k projections + elu+1 feature map:
                #   Q = max(y + 1, min(exp(y), 1)),  y = ps + bias
                QT = [rp.tile([128, seq], BF16, tag=f"QT{t}", name=f"QT{t}") for t in range(2)]
                KTf = [rp.tile([128, seq], BF16, tag=f"KTf{t}", name=f"KTf{t}") for t in range(2)]
                for X, W, be, bp, OUT in (
                    (xq, wq, bqe, bqp, QT),
                    (xk, wk, bke, bkp, KTf),
                ):
                    if X is xk:
                        filler(11)
                    for n in range(nseq):
                        ncols = slice(n * 512, (n + 1) * 512)
                        for mt in range(2):
                            ps = pp1.tile([128, 512], F32, tag="ps", bufs=4)
                            for k in range(KT_N):
                                nc.tensor.matmul(
                                    ps[:],
                                    W[k][:, mt * 128 : (mt + 1) * 128],
                                    X[k][:, ncols],
                                    start=(k == 0),
                                    stop=(k == KT_N - 1),
                                )
                            ex = wp.tile([128, 512], BF16, tag="ex")
                            nc.scalar.activation(ex[:], ps[:], AF.Exp, bias=be[mt][:, 0:1])
                            exc = wp.tile([128, 512], BF16, tag="exc")
                            nc.vector.tensor_scalar_min(exc[:], ex[:], 1.0)
                            nc.vector.scalar_tensor_tensor(
                                OUT[mt][:, ncols], ps[:], bp[mt][:, 0:1], exc[:],
                                op0=OP.add, op1=OP.max,
                            )

                if stop_after == "proj":
                    return nc

                # v projection, natural [pos, ch] layout; +1 ones-column per
                # head for the normalizer state
                vst = [rp.tile([128, HPC * (HD + 1)], BF16, tag=f"vst{m}", name=f"vst{m}") for m in range(seqt)]
                filler(2)
                for m in range(seqt):
                    vv = vst[m].rearrange("p (h e) -> p h e", e=HD + 1)
                    nc.gpsimd.memset(vv[:, :, HD : HD + 1], 1.0)
                    ps2 = pp1.tile([128, CH], F32, tag="ps", bufs=4)
                    for k in range(KT_N):
                        nc.tensor.matmul(
                            ps2[:],
                            xv[k][:, m * 128 : (m + 1) * 128],
                            wv[k][:],
                            start=(k == 0),
                            stop=(k == KT_N - 1),
                        )
                    nc.vector.tensor_copy(
                        vv[:, :, 0:HD],
                        ps2.rearrange("p (h e) -> p h e", e=HD)[:, :, :],
                    )

                if stop_after == "vproj":
                    return nc

            # ---------------- attention phase (own psum pool) ----------------
            # PSUM is bank-granular (8 x 2KB) and matmuls may only write whole
            # tiles (free-dim slice outputs crash the runtime), so the per-head
            # work uses separate small tiles:
            #   at 2 banks, on 2, st 2, tp 1 (k/o transposes), po 1  -> 8
            with tc.tile_pool(name="ps2", bufs=1, space="PSUM") as pp2:
                knat = [rp.tile([128, CH], BF16, tag=f"knat{m}", name=f"knat{m}") for m in range(seqt)]
                onat = [rp.tile([128, CH], BF16, tag=f"onat{m}", name=f"onat{m}") for m in range(seqt)]
                # bf16 running state snapshots, ping-pong; head h lives at
                # partitions (h%2)*64.. and columns (h//2)*65..
                sb = [rp.tile([128, 2 * (HD + 1)], BF16, tag=f"sb{i}", name=f"sb{i}") for i in range(2)]
                OTW = [
                    [rp.tile([128, 512], BF16, tag=f"OTW{n}_{t}", name=f"OTW{n}_{t}") for t in range(2)]
                    for n in range(seqt // 4)
                ]

                def stage_a(j):
                    """A^T = K_c Q_c^T (+ mask) and K natural for chunk j."""
                    cc = slice(j * 128, (j + 1) * 128)
                    atm = wp.tile([128, 512], BF16, tag="atm", name=f"atm{j}")
                    for h in range(HPC):
                        t, off = h // 2, (h % 2) * 64
                        at = pp2.tile([128, 128], F32, tag="at", bufs=2, name=f"at{j}_{h}")
                        nc.tensor.matmul(
                            at[:],
                            KTf[t][off : off + 64, cc],
                            QT[t][off : off + 64, cc],
                            start=True,
                            stop=True,
                        )
                        nc.vector.tensor_tensor(
                            atm[:, h * 128 : (h + 1) * 128],
                            at[:],
                            mask[:, 0:128], op=OP.mult,
                        )
                    for t in range(2):
                        kp = pp2.tile([128, 128], BF16, tag="tp", bufs=1, name=f"kp{j}_{t}")
                        nc.tensor.matmul(
                            kp[:],
                            KTf[t][:, cc],
                            ident[:],
                            is_transpose=True,
                            start=True,
                            stop=True,
                        )
                        nc.scalar.activation(
                            knat[j][:, t * 128 : (t + 1) * 128], kp[:], AF.Copy
                        )
                    return atm

                def stage_c(j):
                    """O transpose for chunk j into its OTW window (DMA XBAR)."""
                    for t in range(2):
                        nc.sync.dma_start(
                            OTW[j // 4][t][:, (j % 4) * 128 : (j % 4 + 1) * 128],
                            onat[j][:, t * 128 : (t + 1) * 128],
                            transpose=True,
                        )

                def _oproj_part(w, part):
                    for jj in range(2 * part, 2 * part + 2):
                        po = pp2.tile([128, 512], F32, tag="po", bufs=2, name=f"po{w}_{jj}")
                        for t in range(2):
                            nc.tensor.matmul(
                                po[:],
                                wo[t][:, jj * 128 : (jj + 1) * 128],
                                OTW[w][t][:],
                                start=(t == 0),
                                stop=(t == 1),
                            )
                        oev = wp.tile([128, 512], BF16, tag="oev", name=f"oev{w}_{jj}")
                        nc.scalar.activation(oev[:], po[:], AF.Copy)
                        nc.sync.dma_start(
                            outT[jj * 128 : (jj + 1) * 128, w * 512 : (w + 1) * 512],
                            oev[:],
                        )

                atm_cur = stage_a(0)
                if stop_after == "stagea":
                    return nc
                stop_iter = (
                    int(stop_after[4:]) if stop_after.startswith("attn") and stop_after != "attn" else None
                )
                for m in range(seqt):
                    if stop_iter is not None and m >= stop_iter:
                        return nc
                    cc = slice(m * 128, (m + 1) * 128)

                    # PE first feeds next chunk's A^T so Vector never starves
                    atm_next = stage_a(m + 1) if m + 1 < seqt else None
                    # chunk m-1's O-transpose, then the spread output
                    # projection: window w is fully transposed after
                    # stage_c(4w+3) just above, so its parts run in
                    # iterations 4w+4 .. 4w+7
                    if m > 0:
                        stage_c(m - 1)
                    if m >= 4:
                        _oproj_part((m - 4) // 4, (m - 4) % 4)

                    # numerator + normalizer, per head: on = A_m Vaug + Q S
                    sbp = sb[(m - 1) % 2]
                    onbs = []
                    for h in range(HPC):
                        t, off = h // 2, (h % 2) * 64
                        cs = slice((h // 2) * (HD + 1), (h // 2 + 1) * (HD + 1))
                        onb = pp2.tile([128, HD + 1], F32, tag="on", bufs=2, name=f"on{m}_{h}")
                        onbs.append(onb)
                        nc.tensor.matmul(
                            onb[:],
                            atm_cur[:, h * 128 : (h + 1) * 128],
                            vst[m][:, h * (HD + 1) : (h + 1) * (HD + 1)],
                            start=True,
                            stop=(m == 0),
                        )
                        if m > 0:
                            nc.tensor.matmul(
                                onb[:],
                                QT[t][off : off + 64, cc],
                                sbp[off : off + 64, cs],
                                start=False,
                                stop=True,
                            )

                    # normalize: onat = numerator * (1/n) (n strictly positive
                    # and >> eps, so the eps add is dropped); reciprocal on
                    # Vector, scaled psum-drain copy on Scalar. Issued BEFORE
                    # the st block: the on-tile bank reuse (bufs=2) makes the
                    # h=2,3 matmuls wait on these drains, which must therefore
                    # not sit behind the fuse ops (which wait on st -> cycle).
                    zrt = wp.tile([128, HPC], F32, tag="zr", bufs=2, name=f"zr{m}")
                    for h in range(HPC):
                        nc.vector.reciprocal(
                            zrt[:, h : h + 1], onbs[h][:, HD : HD + 1]
                        )
                    for h in range(HPC):
                        nc.scalar.activation(
                            onat[m][:, h * HD : (h + 1) * HD],
                            onbs[h][:, 0:HD],
                            AF.Copy,
                            scale=zrt[:, h : h + 1],
                        )

                    # per-head chunk state st = K_c^T Vaug_c, then fused
                    # add+snapshot into the bf16 running state (Vector);
                    # dead for the last chunk
                    if m < seqt - 1:
                        sbt = sb[m % 2]
                        for h in range(HPC):
                            off = (h % 2) * 64
                            cs = slice((h // 2) * (HD + 1), (h // 2 + 1) * (HD + 1))
                            st = pp2.tile([64, HD + 1], F32, tag="st", bufs=1, name=f"st{m}_{h}")
                            nc.tensor.matmul(
                                st[:],
                                knat[m][:, h * HD : (h + 1) * HD],
                                vst[m][:, h * (HD + 1) : (h + 1) * (HD + 1)],
                                start=True,
                                stop=True,
                            )
                            if m == 0:
                                nc.vector.tensor_copy(sbt[off : off + 64, cs], st[:])
                            else:
                                nc.vector.tensor_tensor(
                                    sbt[off : off + 64, cs],
                                    sbp[off : off + 64, cs],
                                    st[:], op=OP.add,
                                )

                    atm_cur = atm_next

                # drain: last chunk's transpose + remaining o-proj parts
                stage_c(seqt - 1)
                done = seqt - 4  # parts issued in-loop
                total = 4 * (seqt // 4)
                for i in range(done, total):
                    _oproj_part(i // 4, i % 4)

                if stop_after == "attn":
                    return nc

    return nc


def _split_multi_waits(nc, max_waits=1):
    """This toolchain's walrus encodes at most one sync-wait per instruction;
    hoist extra waits onto single-wait NoOps on the same engine queue."""
    for f in nc.m.functions:
        for blk in f.blocks:
            insts = list(blk.instructions)
            out, changed = [], False
            for inst in insts:
                si = inst.sync_info
                if si is not None and si.on_wait and len(si.on_wait) > max_waits:
                    waits = list(si.on_wait)
                    hoist, keep = waits[:-max_waits], waits[-max_waits:]
                    for j, w in enumerate(hoist):
                        nop = mybir.InstNoOp(name=f"{inst.name}-ws{j}")
                        nop.engine = inst.engine
                        nop.sync_info = SyncInfo(on_wait=[w], on_update=[])
                        nc.register_instruction(nop)
                        out.append(nop)
                    inst.sync_info = SyncInfo(on_wait=keep, on_update=list(si.on_update))
                    changed = True
                out.append(inst)
            if changed:
                blk.instructions = out


def host_prepare(querys, keys, values, Wq, bq, Wk, bk, Wv, bv, Wo, bo, seq=L):
    """Build the 8 per-core input maps from the full-size fp32 inputs."""
    bf = ml_dtypes.bfloat16
    scale = HD ** -0.5
    mask = np.triu(np.ones((128, 128), np.float32))        # [s, t], keep s <= t
    mask4 = np.tile(mask, (1, 4)).astype(bf)
    ident = np.eye(128, dtype=bf)

    xT = {}
    for b in range(B):
        xT[("q", b)] = np.ascontiguousarray(querys[b, :seq].T).astype(bf)
        xT[("k", b)] = np.ascontiguousarray(keys[b, :seq].T).astype(bf)
        xT[("v", b)] = np.ascontiguousarray(values[b, :seq].T).astype(bf)

    in_maps = []
    for c in range(N_CORES):
        b, g = c // 4, c % 4
        ch = slice(g * CH, (g + 1) * CH)
        bqs = (bq[ch] * scale).astype(np.float32)
        in_maps.append({
            "xqT": xT[("q", b)],
            "xkT": xT[("k", b)],
            "xvT": xT[("v", b)],
            "wqT": np.ascontiguousarray((Wq[ch] * scale).T).astype(bf),
            "wkT": np.ascontiguousarray(Wk[ch].T).astype(bf),
            "wvT": np.ascontiguousarray(Wv[ch].T).astype(bf),
            "woT": np.ascontiguousarray(Wo[:, ch].T).astype(bf),
            "bqe": bqs[:, None],
            "bqp": (bqs + 1.0)[:, None],
            "bke": bk[ch].astype(np.float32)[:, None],
            "bkp": (bk[ch] + 1.0).astype(np.float32)[:, None],
            "mask4": mask4,
            "ident": ident,
        })
    return in_maps


def gather_output(results, bo2, seq=L):
    """Sum per-core bf16 transposed partials in fp32, transpose, add bias."""
    out = np.empty((B, seq, D_MODEL), np.float32)
    for b in range(B):
        acc = results[4 * b]["outT"].astype(np.float32)
        for g in range(1, 4):
            acc += results[4 * b + g]["outT"].astype(np.float32)
        out[b] = acc.T + bo2[None, :]
    return out


_nc_cache = {}


def kernel(**inputs):
    from concourse.bass_utils import run_bass_kernel_spmd

    if L not in _nc_cache:
        _nc_cache[L] = build_nc(L)
    nc = _nc_cache[L]
    in_maps = host_prepare(**inputs)
    res = run_bass_kernel_spmd(nc, in_maps, list(range(N_CORES)))
    bo2 = (
        np.asarray(inputs["bo"], np.float64)
        + np.asarray(inputs["Wo"], np.float64) @ np.asarray(inputs["bv"], np.float64)
    ).astype(np.float32)
    return gather_output([res.results[c] for c in range(N_CORES)], bo2)


# revision 23
# speedup vs baseline: 1.3044x; 1.1340x over previous
"""Causal linear attention layer (elu+1 feature map) on 8 Trainium2 NeuronCores.

Sharding: batch x head-group parallel. 32 (batch, head) jobs -> 8 cores:
core c handles batch b = c // 4 and heads 4*(c%4) .. 4*(c%4)+3, i.e. a
256-channel slice of the projections. Each core:
  - computes its slice of the q/k/v projections (contraction over the full
    d_model, weights pre-sliced + pre-transposed on host, bf16),
  - applies the elu+1 feature map to q, k,
  - runs chunked causal linear attention (chunk = 128 positions) with a
    per-head [64, 64+1] fp32 state resident in PSUM (the +1 column carries
    the running sum of K for the normalizer),
  - projects through the matching 256-column slice of Wo, writing a
    transposed partial output [1024, seq] bf16.
Host sums the 4 partials per batch, transposes, and adds bo' = bo + Wo@bv
(the v bias passes through the normalized attention exactly, so it is
folded into the output bias and dropped from the device-side v projection).

Performance notes (vs the previous version):
  - input DMAs are issued in consumption order with xq/xk column-split so
    the first projection matmuls start ~4us in instead of ~37us,
  - dead "warmup" matmuls keep the PE busy during the initial load so the
    DVFS p-state ramps before real work arrives,
  - the attention chunk loop is software-pipelined: A^T/K-transpose for
    chunk m+1 and O-transpose/output-projection for chunk m-1 are issued
    inside iteration m so the PE never sits on the serial state chain,
  - the K^T V state accumulates directly in a PSUM bank across all chunks
    (no vector adds); bf16 snapshots of the state feed the Q@S matmuls,
  - elementwise work is spread across Vector, Scalar and GpSimd engines,
  - output partials are stored bf16 (host sums in fp32).
"""

import numpy as np
import ml_dtypes

import concourse.bass as bass
import concourse.mybir as mybir
import concourse.tile as tile
from bass_rust import SyncInfo

BF16 = mybir.dt.bfloat16
F32 = mybir.dt.float32
AF = mybir.ActivationFunctionType
OP = mybir.AluOpType

D_MODEL = 1024
N_HEAD = 16
HD = 64
B = 2
L = 2048
N_CORES = 8
HPC = 4                 # heads per core
CH = HPC * HD           # 256 channel slice per core
KT_N = D_MODEL // 128   # 8 k-tiles for the projections
CHUNK = 128
NWARM = 36              # dead matmuls to ramp the PE while inputs stream in
NFILL = 5               # dead matmuls at phase boundaries (keep the PE pipeline warm)


def build_nc(seq=L, stop_after="all"):
    nc = _build_nc_inner(seq, stop_after)
    _split_multi_waits(nc)
    return nc


def _build_nc_inner(seq=L, stop_after="all"):
    """Build the per-core Bass program (SPMD: all 8 cores run this)."""
    assert seq % 512 == 0
    seqt = seq // CHUNK          # chunks / seq tiles
    nseq = seq // 512            # 512-wide column chunks for projections

    nc = bass.Bass("TRN2", target_bir_lowering=False)

    xqT = nc.dram_tensor("xqT", [D_MODEL, seq], BF16, kind="ExternalInput")
    xkT = nc.dram_tensor("xkT", [D_MODEL, seq], BF16, kind="ExternalInput")
    xvT = nc.dram_tensor("xvT", [D_MODEL, seq], BF16, kind="ExternalInput")
    wqT = nc.dram_tensor("wqT", [D_MODEL, CH], BF16, kind="ExternalInput")
    wkT = nc.dram_tensor("wkT", [D_MODEL, CH], BF16, kind="ExternalInput")
    wvT = nc.dram_tensor("wvT", [D_MODEL, CH], BF16, kind="ExternalInput")
    woT = nc.dram_tensor("woT", [CH, D_MODEL], BF16, kind="ExternalInput")
    bqe_d = nc.dram_tensor("bqe", [CH, 1], F32, kind="ExternalInput")   # scaled bq
    bqp_d = nc.dram_tensor("bqp", [CH, 1], F32, kind="ExternalInput")   # scaled bq + 1
    bke_d = nc.dram_tensor("bke", [CH, 1], F32, kind="ExternalInput")
    bkp_d = nc.dram_tensor("bkp", [CH, 1], F32, kind="ExternalInput")
    mask_d = nc.dram_tensor("mask4", [128, 512], BF16, kind="ExternalInput")  # causal mask, 4x tiled
    ident_d = nc.dram_tensor("ident", [128, 128], BF16, kind="ExternalInput")
    outT = nc.dram_tensor("outT", [D_MODEL, seq], BF16, kind="ExternalOutput")

    with tile.TileContext(nc) as tc:
        with (
            tc.tile_pool(name="res", bufs=1) as rp,
            tc.tile_pool(name="work", bufs=3) as wp,
        ):
            # ---------------- constants first (warmup operands) ----------------
            mask = rp.tile([128, 512], BF16, tag="mask")
            nc.sync.dma_start(mask[:], mask_d[:])
            ident = rp.tile([128, 128], BF16, tag="ident")
            nc.sync.dma_start(ident[:], ident_d[:])

            # ---------------- weight / bias / activation DMAs ----------------
            # Issued in consumption order so compute can chase the DMA queue.
            wq = [rp.tile([128, CH], BF16, tag=f"wq{k}", name=f"wq{k}") for k in range(KT_N)]
            wk = [rp.tile([128, CH], BF16, tag=f"wk{k}", name=f"wk{k}") for k in range(KT_N)]
            wv = [rp.tile([128, CH], BF16, tag=f"wv{k}", name=f"wv{k}") for k in range(KT_N)]
            wo = [rp.tile([128, D_MODEL], BF16, tag=f"wo{t}", name=f"wo{t}") for t in range(2)]
            bqe, bqp, bke, bkp = (
                [rp.tile([128, 1], F32, tag=f"b{i}{t}", name=f"b{i}{t}") for t in range(2)]
                for i in range(4)
            )
            xq = [rp.tile([128, seq], BF16, tag=f"xq{k}", name=f"xq{k}") for k in range(KT_N)]
            xk = [rp.tile([128, seq], BF16, tag=f"xk{k}", name=f"xk{k}") for k in range(KT_N)]
            xv = [rp.tile([128, seq], BF16, tag=f"xv{k}", name=f"xv{k}") for k in range(KT_N)]

            for k in range(KT_N):
                nc.sync.dma_start(wq[k][:], wqT[k * 128 : (k + 1) * 128, :])
            for t in range(2):
                sl = slice(t * 128, (t + 1) * 128)
                nc.sync.dma_start(bqe[t][:], bqe_d[sl, :])
                nc.sync.dma_start(bqp[t][:], bqp_d[sl, :])
                nc.sync.dma_start(bke[t][:], bke_d[sl, :])
                nc.sync.dma_start(bkp[t][:], bkp_d[sl, :])
            # full-row tile loads: per-DMA fixed cost (~600ns) makes narrow
            # column-split transfers inefficient
            for k in range(KT_N):
                nc.sync.dma_start(xq[k][:], xqT[k * 128 : (k + 1) * 128, :])
            for k in range(KT_N):
                nc.sync.dma_start(wk[k][:], wkT[k * 128 : (k + 1) * 128, :])
            for k in range(KT_N):
                nc.sync.dma_start(xk[k][:], xkT[k * 128 : (k + 1) * 128, :])
            for k in range(KT_N):
                nc.sync.dma_start(wv[k][:], wvT[k * 128 : (k + 1) * 128, :])
            for k in range(KT_N):
                nc.sync.dma_start(xv[k][:], xvT[k * 128 : (k + 1) * 128, :])
            for t in range(2):
                nc.sync.dma_start(wo[t][:], woT[t * 128 : (t + 1) * 128, :])

            # ---------------- projection phase (own psum pool) ----------------
            with tc.tile_pool(name="ps1", bufs=1, space="PSUM") as pp1:
                # dead warmup matmuls: keep the PE streaming while DMAs land
                for i in range(NWARM):
                    ps = pp1.tile([128, 512], F32, tag="ps", bufs=4)
                    nc.tensor.matmul(ps[:], mask[:, 0:128], mask[:], start=True, stop=True)

                def filler(k=NFILL):
                    for _ in range(k):
                        fps = pp1.tile([128, 512], F32, tag="ps", bufs=4)
                        nc.tensor.matmul(fps[:], mask[:, 0:128], mask[:], start=True, stop=True)

                # q/k projections + elu+1 feature map:
                #   Q = max(y + 1, min(exp(y), 1)),  y = ps + bias
                QT = [rp.tile([128, seq], BF16, tag=f"QT{t}", name=f"QT{t}") for t in range(2)]
                KTf = [rp.tile([128, seq], BF16, tag=f"KTf{t}", name=f"KTf{t}") for t in range(2)]
                for X, W, be, bp, OUT in (
                    (xq, wq, bqe, bqp, QT),
                    (xk, wk, bke, bkp, KTf),
                ):
                    if X is xk:
                        filler(11)
                    for n in range(nseq):
                        ncols = slice(n * 512, (n + 1) * 512)
                        for mt in range(2):
                            ps = pp1.tile([128, 512], F32, tag="ps", bufs=4)
                            for k in range(KT_N):
                                nc.tensor.matmul(
                                    ps[:],
                                    W[k][:, mt * 128 : (mt + 1) * 128],
                                    X[k][:, ncols],
                                    start=(k == 0),
                                    stop=(k == KT_N - 1),
                                )
                            ex = wp.tile([128, 512], BF16, tag="ex")
                            nc.scalar.activation(ex[:], ps[:], AF.Exp, bias=be[mt][:, 0:1])
                            exc = wp.tile([128, 512], BF16, tag="exc")
                            nc.vector.tensor_scalar_min(exc[:], ex[:], 1.0)
                            nc.vector.scalar_tensor_tensor(
                                OUT[mt][:, ncols], ps[:], bp[mt][:, 0:1], exc[:],
                                op0=OP.add, op1=OP.max,
                            )

                if stop_after == "proj":
                    return nc

                # v projection, natural [pos, ch] layout; +1 ones-column per
                # head for the normalizer state
                vst = [rp.tile([128, HPC * (HD + 1)], BF16, tag=f"vst{m}", name=f"vst{m}") for m in range(seqt)]
                filler(2)
                for m in range(seqt):
                    vv = vst[m].rearrange("p (h e) -> p h e", e=HD + 1)
                    nc.gpsimd.memset(vv[:, :, HD : HD + 1], 1.0)
                    ps2 = pp1.tile([128, CH], F32, tag="ps", bufs=4)
                    for k in range(KT_N):
                        nc.tensor.matmul(
                            ps2[:],
                            xv[k][:, m * 128 : (m + 1) * 128],
                            wv[k][:],
                            start=(k == 0),
                            stop=(k == KT_N - 1),
                        )
                    nc.vector.tensor_copy(
                        vv[:, :, 0:HD],
                        ps2.rearrange("p (h e) -> p h e", e=HD)[:, :, :],
                    )

                if stop_after == "vproj":
                    return nc

            # ---------------- attention phase (own psum pool) ----------------
            # PSUM is bank-granular (8 x 2KB) and matmuls may only write whole
            # tiles (free-dim slice outputs crash the runtime), so the per-head
            # work uses separate small tiles:
            #   at 2 banks, on 2, st 2, tp 1 (k/o transposes), po 1  -> 8
            with tc.tile_pool(name="ps2", bufs=1, space="PSUM") as pp2:
                knat = [rp.tile([128, CH], BF16, tag=f"knat{m}", name=f"knat{m}") for m in range(seqt)]
                onat = [rp.tile([128, CH], BF16, tag=f"onat{m}", name=f"onat{m}") for m in range(seqt)]
                # bf16 running state snapshots, ping-pong; head h lives at
                # partitions (h%2)*64.. and columns (h//2)*65..
                sb = [rp.tile([128, 2 * (HD + 1)], BF16, tag=f"sb{i}", name=f"sb{i}") for i in range(2)]
                OTW = [
                    [rp.tile([128, 512], BF16, tag=f"OTW{n}_{t}", name=f"OTW{n}_{t}") for t in range(2)]
                    for n in range(seqt // 4)
                ]

                def stage_a(j):
                    """A^T = K_c Q_c^T (+ mask) and K natural for chunk j."""
                    cc = slice(j * 128, (j + 1) * 128)
                    atm = wp.tile([128, 512], BF16, tag="atm", name=f"atm{j}")
                    for h in range(HPC):
                        t, off = h // 2, (h % 2) * 64
                        at = pp2.tile([128, 128], F32, tag="at", bufs=2, name=f"at{j}_{h}")
                        nc.tensor.matmul(
                            at[:],
                            KTf[t][off : off + 64, cc],
                            QT[t][off : off + 64, cc],
                            start=True,
                            stop=True,
                        )
                        nc.vector.tensor_tensor(
                            atm[:, h * 128 : (h + 1) * 128],
                            at[:],
                            mask[:, 0:128], op=OP.mult,
                        )
                    for t in range(2):
                        kp = pp2.tile([128, 128], BF16, tag="tp", bufs=1, name=f"kp{j}_{t}")
                        nc.tensor.matmul(
                            kp[:],
                            KTf[t][:, cc],
                            ident[:],
                            is_transpose=True,
                            start=True,
                            stop=True,
                        )
                        nc.scalar.activation(
                            knat[j][:, t * 128 : (t + 1) * 128], kp[:], AF.Copy
                        )
                    return atm

                def stage_c(j):
                    """O transpose for chunk j into its OTW window."""
                    for t in range(2):
                        otp = pp2.tile([128, 128], BF16, tag="tp", bufs=1, name=f"otp{j}_{t}")
                        nc.tensor.matmul(
                            otp[:],
                            onat[j][:, t * 128 : (t + 1) * 128],
                            ident[:],
                            is_transpose=True,
                            start=True,
                            stop=True,
                        )
                        nc.vector.tensor_copy(
                            OTW[j // 4][t][:, (j % 4) * 128 : (j % 4 + 1) * 128],
                            otp[:],
                        )

                def _oproj_part(w, part):
                    for jj in range(2 * part, 2 * part + 2):
                        po = pp2.tile([128, 512], F32, tag="po", bufs=2, name=f"po{w}_{jj}")
                        for t in range(2):
                            nc.tensor.matmul(
                                po[:],
                                wo[t][:, jj * 128 : (jj + 1) * 128],
                                OTW[w][t][:],
                                start=(t == 0),
                                stop=(t == 1),
                            )
                        oev = wp.tile([128, 512], BF16, tag="oev", name=f"oev{w}_{jj}")
                        nc.scalar.activation(oev[:], po[:], AF.Copy)
                        nc.sync.dma_start(
                            outT[jj * 128 : (jj + 1) * 128, w * 512 : (w + 1) * 512],
                            oev[:],
                        )

                atm_cur = stage_a(0)
                if stop_after == "stagea":
                    return nc
                stop_iter = (
                    int(stop_after[4:]) if stop_after.startswith("attn") and stop_after != "attn" else None
                )
                for m in range(seqt):
                    if stop_iter is not None and m >= stop_iter:
                        return nc
                    cc = slice(m * 128, (m + 1) * 128)

                    # PE first feeds next chunk's A^T so Vector never starves
                    atm_next = stage_a(m + 1) if m + 1 < seqt else None
                    # chunk m-1's O-transpose, then the spread output
                    # projection: window w is fully transposed after
                    # stage_c(4w+3) just above, so its parts run in
                    # iterations 4w+4 .. 4w+7
                    if m > 0:
                        stage_c(m - 1)
                    if m >= 4:
                        _oproj_part((m - 4) // 4, (m - 4) % 4)

                    # numerator + normalizer, per head: on = A_m Vaug + Q S
                    sbp = sb[(m - 1) % 2]
                    onbs = []
                    for h in range(HPC):
                        t, off = h // 2, (h % 2) * 64
                        cs = slice((h // 2) * (HD + 1), (h // 2 + 1) * (HD + 1))
                        onb = pp2.tile([128, HD + 1], F32, tag="on", bufs=2, name=f"on{m}_{h}")
                        onbs.append(onb)
                        nc.tensor.matmul(
                            onb[:],
                            atm_cur[:, h * 128 : (h + 1) * 128],
                            vst[m][:, h * (HD + 1) : (h + 1) * (HD + 1)],
                            start=True,
                            stop=(m == 0),
                        )
                        if m > 0:
                            nc.tensor.matmul(
                                onb[:],
                                QT[t][off : off + 64, cc],
                                sbp[off : off + 64, cs],
                                start=False,
                                stop=True,
                            )

                    # normalize: onat = numerator * (1/n) (n strictly positive
                    # and >> eps, so the eps add is dropped); reciprocal on
                    # Vector, scaled psum-drain copy on Scalar. Issued BEFORE
                    # the st block: the on-tile bank reuse (bufs=2) makes the
                    # h=2,3 matmuls wait on these drains, which must therefore
                    # not sit behind the fuse ops (which wait on st -> cycle).
                    zrt = wp.tile([128, HPC], F32, tag="zr", bufs=2, name=f"zr{m}")
                    for h in range(HPC):
                        nc.vector.reciprocal(
                            zrt[:, h : h + 1], onbs[h][:, HD : HD + 1]
                        )
                    for h in range(HPC):
                        nc.scalar.activation(
                            onat[m][:, h * HD : (h + 1) * HD],
                            onbs[h][:, 0:HD],
                            AF.Copy,
                            scale=zrt[:, h : h + 1],
                        )

                    # per-head chunk state st = K_c^T Vaug_c, then fused
                    # add+snapshot into the bf16 running state (Vector);
                    # dead for the last chunk
                    if m < seqt - 1:
                        sbt = sb[m % 2]
                        for h in range(HPC):
                            off = (h % 2) * 64
                            cs = slice((h // 2) * (HD + 1), (h // 2 + 1) * (HD + 1))
                            st = pp2.tile([64, HD + 1], F32, tag="st", bufs=1, name=f"st{m}_{h}")
                            nc.tensor.matmul(
                                st[:],
                                knat[m][:, h * HD : (h + 1) * HD],
                                vst[m][:, h * (HD + 1) : (h + 1) * (HD + 1)],
                                start=True,
                                stop=True,
                            )
                            if m == 0:
                                nc.vector.tensor_copy(sbt[off : off + 64, cs], st[:])
                            else:
                                nc.vector.tensor_tensor(
                                    sbt[off : off + 64, cs],
                                    sbp[off : off + 64, cs],
                                    st[:], op=OP.add,
                                )

                    atm_cur = atm_next

                # drain: last chunk's transpose + remaining o-proj parts
                stage_c(seqt - 1)
                done = seqt - 4  # parts issued in-loop
                total = 4 * (seqt // 4)
                for i in range(done, total):
                    _oproj_part(i // 4, i % 4)

                if stop_after == "attn":
                    return nc

    return nc


def _split_multi_waits(nc, max_waits=1):
    """This toolchain's walrus encodes at most one sync-wait per instruction;
    hoist extra waits onto single-wait NoOps on the same engine queue."""
    for f in nc.m.functions:
        for blk in f.blocks:
            insts = list(blk.instructions)
            out, changed = [], False
            for inst in insts:
                si = inst.sync_info
                if si is not None and si.on_wait and len(si.on_wait) > max_waits:
                    waits = list(si.on_wait)
                    hoist, keep = waits[:-max_waits], waits[-max_waits:]
                    for j, w in enumerate(hoist):
                        nop = mybir.InstNoOp(name=f"{inst.name}-ws{j}")
                        nop.engine = inst.engine
                        nop.sync_info = SyncInfo(on_wait=[w], on_update=[])
                        nc.register_instruction(nop)
                        out.append(nop)
                    inst.sync_info = SyncInfo(on_wait=keep, on_update=list(si.on_update))
                    changed = True
                out.append(inst)
            if changed:
                blk.instructions = out


def host_prepare(querys, keys, values, Wq, bq, Wk, bk, Wv, bv, Wo, bo, seq=L):
    """Build the 8 per-core input maps from the full-size fp32 inputs."""
    bf = ml_dtypes.bfloat16
    scale = HD ** -0.5
    mask = np.triu(np.ones((128, 128), np.float32))        # [s, t], keep s <= t
    mask4 = np.tile(mask, (1, 4)).astype(bf)
    ident = np.eye(128, dtype=bf)

    xT = {}
    for b in range(B):
        xT[("q", b)] = np.ascontiguousarray(querys[b, :seq].T).astype(bf)
        xT[("k", b)] = np.ascontiguousarray(keys[b, :seq].T).astype(bf)
        xT[("v", b)] = np.ascontiguousarray(values[b, :seq].T).astype(bf)

    in_maps = []
    for c in range(N_CORES):
        b, g = c // 4, c % 4
        ch = slice(g * CH, (g + 1) * CH)
        bqs = (bq[ch] * scale).astype(np.float32)
        in_maps.append({
            "xqT": xT[("q", b)],
            "xkT": xT[("k", b)],
            "xvT": xT[("v", b)],
            "wqT": np.ascontiguousarray((Wq[ch] * scale).T).astype(bf),
            "wkT": np.ascontiguousarray(Wk[ch].T).astype(bf),
            "wvT": np.ascontiguousarray(Wv[ch].T).astype(bf),
            "woT": np.ascontiguousarray(Wo[:, ch].T).astype(bf),
            "bqe": bqs[:, None],
            "bqp": (bqs + 1.0)[:, None],
            "bke": bk[ch].astype(np.float32)[:, None],
            "bkp": (bk[ch] + 1.0).astype(np.float32)[:, None],
            "mask4": mask4,
            "ident": ident,
        })
    return in_maps


def gather_output(results, bo2, seq=L):
    """Sum per-core bf16 transposed partials in fp32, transpose, add bias."""
    out = np.empty((B, seq, D_MODEL), np.float32)
    for b in range(B):
        acc = results[4 * b]["outT"].astype(np.float32)
        for g in range(1, 4):
            acc += results[4 * b + g]["outT"].astype(np.float32)
        out[b] = acc.T + bo2[None, :]
    return out


_nc_cache = {}


def kernel(**inputs):
    from concourse.bass_utils import run_bass_kernel_spmd

    if L not in _nc_cache:
        _nc_cache[L] = build_nc(L)
    nc = _nc_cache[L]
    in_maps = host_prepare(**inputs)
    res = run_bass_kernel_spmd(nc, in_maps, list(range(N_CORES)))
    bo2 = (
        np.asarray(inputs["bo"], np.float64)
        + np.asarray(inputs["Wo"], np.float64) @ np.asarray(inputs["bv"], np.float64)
    ).astype(np.float32)
    return gather_output([res.results[c] for c in range(N_CORES)], bo2)
